# revision 32
# baseline (speedup 1.0000x reference)
"""Self-contained Trainium2 Bass kernel for nn_GRUModel_16569983828350.

2-layer GRU, B=128, T=1000, I=64, H=512, head -> sigmoid [128, 1].
Sharding: data-parallel over batch across 8 NeuronCores (16 rows/core);
weights replicated; no cross-core communication. Feature-major on-chip
layout (gate/hidden features on SBUF partitions, batch on the free dim).

Latency-oriented recurrence step (the 1000-step serial chain dominates):
  - x-projection and b_hn are injected into the gate PSUM banks by the PE
    itself (identity / rank-1 ones matmuls), removing two vector adds from
    the serial chain.
  - sigmoid reads PSUM directly; gate math is bf16 end to end.
  - n_neg = tanh(-pren) plus a fused scalar_tensor_tensor
    (z - 1) * n_neg == (1 - z) * n shortens the post-tanh chain to two
    same-engine vector ops; z*h_prev is precomputed during the tanh.
  - elementwise work is split across DVE (nc.vector) and Pool (nc.gpsimd).
  - bulk x-projection matmuls/evictions are software-interleaved into the
    recurrence instruction stream (no serial xproj bursts; keeps the PE
    p-state high), double-buffered across blocks of L=100 timesteps.
  - the two layers' recurrences are interleaved (layer 1 lags one block).
"""

import numpy as np

import concourse.bass as bass
import concourse.mybir as mybir
import concourse.tile as tile
from concourse.vector_clock import ScopedClock

MAX_WAITS_PER_INST = 1

def _patched_drain_and_barrier(self, tick_clock, wait_clock):
    carrier = self.nc.sync.nop(nofuse=True, hint="drain_wait_carrier")
    wait_clock.add_sem_waits(
        carrier.ins, ScopedClock({None: tick_clock.global_clock})
    )
    si = carrier.ins.sync_info
    if si is not None and si.on_wait and len(si.on_wait) > MAX_WAITS_PER_INST:
        waits = list(si.on_wait)
        carrier.ins.sync_info = mybir.SyncInfo(
            on_wait=waits[:MAX_WAITS_PER_INST], on_update=list(si.on_update)
        )
        for i in range(MAX_WAITS_PER_INST, len(waits), MAX_WAITS_PER_INST):
            w = self.nc.sync.nop(nofuse=True, hint="drain_wait_spill")
            w.ins.sync_info = mybir.SyncInfo(
                on_wait=waits[i : i + MAX_WAITS_PER_INST], on_update=[]
            )

    self.nc.sync.drain()
    self.nc.all_engine_barrier()
    assert self.sems is not None
    popped = self.nc._tile_sem_poison_stack.pop()
    assert popped is self._sem_poison
    self.nc.clear_and_free_semaphores(list(self.sems.allocated().values()))
    self.nc.all_engine_barrier()


def split_excess_waits(nc, max_waits: int = 1):
    """Post-pass: any instruction with >max_waits sem waits gets the excess
    moved onto preceding NoOps on the same engine (FIFO order preserves
    semantics). Works around this walrus build's per-instruction wait-slot
    limit."""
    for fn in nc.m.functions:
        for bb in fn.blocks:
            insts = bb.instructions
            out = []
            for inst in insts:
                si = inst.sync_info
                if si is not None and si.on_wait and len(si.on_wait) > max_waits:
                    waits = list(si.on_wait)
                    keep = waits[:max_waits]
                    rest = waits[max_waits:]
                    for j in range(0, len(rest), max_waits):
                        nop = mybir.InstNoOp(
                            name=f"{inst.name}-wsp{j}", ins=[], outs=[]
                        )
                        nop.engine = inst.engine
                        nop.sync_info = mybir.SyncInfo(
                            on_wait=rest[j : j + max_waits], on_update=[]
                        )
                        out.append(nop)
                    inst.sync_info = mybir.SyncInfo(
                        on_wait=keep, on_update=list(si.on_update)
                    )
                out.append(inst)
            if len(out) != len(insts):
                bb.instructions = out


FP32 = mybir.dt.float32
BF16 = mybir.dt.bfloat16
AF = mybir.ActivationFunctionType
ALU = mybir.AluOpType

H = 512
I_IN = 64
G3 = 3 * H  # 1536
KH = H // 128  # 4 k-chunks of hidden
M3 = G3 // 128  # 12 m-tiles of gates


def build_gru_nc(B: int, T: int, L: int, split_waits: bool = True,
                 debug: bool = False):
    """Returns nc. B = per-core batch, T = seq len, L = time-block length."""
    assert T % L == 0
    NB = T // L
    BL = B * L
    NXC = 4  # xproj chunks per block
    NCH = BL // NXC  # xproj psum chunk width (400 <= 512 fp32 psum bank)
    SCH = NCH // B  # timesteps per xproj chunk (25)

    nc = bass.Bass()

    # ---- DRAM I/O (host pre-arranges layouts; see host_pack_inputs) ----
    xT = nc.declare_dram_parameter("xT", [I_IN, T * B], BF16, isOutput=False)
    wih0 = nc.declare_dram_parameter("wih0", [I_IN, G3], BF16, isOutput=False)
    whh0 = nc.declare_dram_parameter("whh0", [128, KH * G3], BF16, isOutput=False)
    wih1 = nc.declare_dram_parameter("wih1", [128, KH * G3], BF16, isOutput=False)
    whh1 = nc.declare_dram_parameter("whh1", [128, KH * G3], BF16, isOutput=False)
    brz0 = nc.declare_dram_parameter("brz0", [128, 8], FP32, isOutput=False)
    bn0 = nc.declare_dram_parameter("bn0", [128, 4], FP32, isOutput=False)
    brz1 = nc.declare_dram_parameter("brz1", [128, 8], FP32, isOutput=False)
    bn1 = nc.declare_dram_parameter("bn1", [128, 4], FP32, isOutput=False)
    bhn0r = nc.declare_dram_parameter("bhn0r", [1, H], BF16, isOutput=False)
    bhn1r = nc.declare_dram_parameter("bhn1r", [1, H], BF16, isOutput=False)
    ident = nc.declare_dram_parameter("ident", [128, 128], BF16, isOutput=False)
    onesb = nc.declare_dram_parameter("onesb", [KH, KH * B], BF16, isOutput=False)
    wfc = nc.declare_dram_parameter("wfc", [128, KH], BF16, isOutput=False)
    bfc = nc.declare_dram_parameter("bfc", [1, 1], FP32, isOutput=False)
    out = nc.declare_dram_parameter("out", [1, B], FP32, isOutput=True)
    if debug:
        h0dbg = nc.declare_dram_parameter(
            "h0dbg", [128, L * 4 * B], BF16, isOutput=True)
        h1dbg = nc.declare_dram_parameter(
            "h1dbg", [128, 4 * B], BF16, isOutput=True)
        xpdbg = nc.declare_dram_parameter(
            "xpdbg", [128, L * 8 * B], BF16, isOutput=True)
        xpndbg = nc.declare_dram_parameter(
            "xpndbg", [128, L * 4 * B], BF16, isOutput=True)
        s0dbg = nc.declare_dram_parameter(
            "s0dbg", [128, 40 * B], FP32, isOutput=True)

    with tile.TileContext(nc) as tc:
        with (
            tc.tile_pool(name="persist", bufs=1) as pp,
            tc.tile_pool(name="xblkp", bufs=2) as xbp,
            tc.tile_pool(name="work", bufs=3) as wp,
            tc.tile_pool(name="gpsum", bufs=2, space="PSUM") as gp,
            tc.tile_pool(name="xpsum", bufs=3, space="PSUM") as xp_ps,
        ):
            # ---- persistent SBUF tiles ----
            wih0_sb = pp.tile([I_IN, G3], BF16, tag="wih0")
            whh0_sb = pp.tile([128, KH * G3], BF16, tag="whh0")
            wih1_sb = pp.tile([128, KH * G3], BF16, tag="wih1")
            whh1_sb = pp.tile([128, KH * G3], BF16, tag="whh1")
            brz0_sb = pp.tile([128, 8], FP32, tag="brz0")
            bn0_sb = pp.tile([128, 4], FP32, tag="bn0")
            brz1_sb = pp.tile([128, 8], FP32, tag="brz1")
            bn1_sb = pp.tile([128, 4], FP32, tag="bn1")
            bhn0_sb = pp.tile([1, H], BF16, tag="bhn0r")
            bhn1_sb = pp.tile([1, H], BF16, tag="bhn1r")
            ident_sb = pp.tile([128, 128], BF16, tag="ident")
            ones_bd = pp.tile([KH, KH * B], BF16, tag="onesb")
            wfc_sb = pp.tile([128, KH], BF16, tag="wfc")
            bfc_sb = pp.tile([1, 1], FP32, tag="bfc")

            xp_rzA = pp.tile([128, L, 8 * B], BF16, tag="xp_rzA")
            xp_nA = pp.tile([128, L, 4 * B], BF16, tag="xp_nA")
            xp_rzB = pp.tile([128, L, 8 * B], BF16, tag="xp_rzB")
            xp_nB = pp.tile([128, L, 4 * B], BF16, tag="xp_nB")
            h0seq = pp.tile([128, L + 1, 4 * B], BF16, tag="h0seq")
            h1bf = pp.tile([128, 4 * B], BF16, tag="h1bf")

            for sb, dram in [
                (wih0_sb, wih0), (whh0_sb, whh0), (wih1_sb, wih1),
                (whh1_sb, whh1), (brz0_sb, brz0), (bn0_sb, bn0),
                (brz1_sb, brz1), (bn1_sb, bn1), (bhn0_sb, bhn0r),
                (bhn1_sb, bhn1r), (ident_sb, ident), (ones_bd, onesb),
                (wfc_sb, wfc), (bfc_sb, bfc),
            ]:
                nc.sync.dma_start(sb[:], dram[:])

            nc.vector.memset(h1bf[:], 0.0)
            nc.vector.memset(h0seq[:, L], 0.0)

            # ---------------- xproj job emitters ----------------
            # One job = one (m-tile, chunk): matmuls into a psum + eviction
            # (bias fused) into the xp SBUF tile. ch covers block timesteps
            # [ch*SCH, (ch+1)*SCH).
            def xproj_mm_l0(xblk, ch, m, ps):
                nc.tensor.matmul(
                    ps[:],
                    wih0_sb[:, m * 128 : (m + 1) * 128],
                    xblk[:, ch * NCH : (ch + 1) * NCH],
                    start=True, stop=True,
                )

            def xproj_mm_l1(ch, m, ps):
                t0 = ch * SCH
                for ki in range(KH):
                    nc.tensor.matmul(
                        ps[:],
                        wih1_sb[:, ki * G3 + m * 128 : ki * G3 + (m + 1) * 128],
                        h0seq[:, t0 + 1 : t0 + 1 + SCH, ki * B : (ki + 1) * B],
                        start=(ki == 0), stop=(ki == KH - 1),
                    )

            def xproj_ev(ch, m, ps, b_rz, b_n, dst_rz, dst_n, on_act):
                t0 = ch * SCH
                if m < 8:
                    dst = dst_rz[:, t0 : t0 + SCH, m * B : (m + 1) * B]
                    bias = b_rz[:, m : m + 1]
                else:
                    dst = dst_n[:, t0 : t0 + SCH, (m - 8) * B : (m - 7) * B]
                    bias = b_n[:, m - 8 : m - 7]
                psv = ps[:].rearrange("p (t b) -> p t b", b=B)
                if on_act:
                    nc.scalar.activation(dst, psv, AF.Identity, bias=bias)
                else:
                    nc.vector.tensor_scalar(dst, psv, bias, None, op0=ALU.add)

            # Per-step job scheduler: sched[tl] -> list of (kind, ch, m)
            # kind 0 = L0-xproj (needs xblk of the NEXT block),
            # kind 1 = L1-xproj of the current block's h0seq.
            # Chunk ch jobs run at steps >= ch*SCH + SCH + 1 (after the xp_A
            # slots they overwrite were read / h0seq slots written); chunk
            # NXC-1 is deferred into the next block's steps [1, SCH].
            def make_sched(have_l0, have_l1, have_l0_tail, have_l1_tail):
                sched = [[] for _ in range(L)]

                def spread(kind, ch, s0, s1):
                    # 12 jobs over steps [s0, s1)
                    steps = list(range(s0, s1))
                    jobs = [(kind, ch, m) for m in range(M3)]
                    for j, job in enumerate(jobs):
                        sched[steps[(j * len(steps)) // len(jobs)]].append(job)

                # tails (chunk NXC-1 of the previous block's work)
                if have_l0_tail:
                    spread(0, NXC - 1, 1, SCH)
                if have_l1_tail:
                    spread(3, NXC - 1, 1, SCH)
                for ch in range(NXC - 1):
                    s0 = ch * SCH + SCH + 1
                    s1 = s0 + SCH - 2
                    if have_l0:
                        spread(0, ch, s0, s1)
                    if have_l1:
                        spread(1, ch, s0, s1)
                return sched

            ev_flip = [0]

            def run_job(kind, ch, m, xblk_next):
                # kind 0: L0-xproj for next block (tail: this block) -> A bufs
                # kind 1/3: L1-xproj of current/prev block h0seq -> B bufs
                on_act = bool(ev_flip[0])
                ev_flip[0] ^= 1
                ps = xp_ps.tile([128, NCH], FP32, tag="xps")
                if kind in (0, 2):
                    xproj_mm_l0(xblk_next, ch, m, ps)
                    xproj_ev(ch, m, ps, brz0_sb, bn0_sb, xp_rzA, xp_nA, on_act)
                else:
                    xproj_mm_l1(ch, m, ps)
                    xproj_ev(ch, m, ps, brz1_sb, bn1_sb, xp_rzB, xp_nB, on_act)

            # ---------------- recurrence step ----------------
            def gate_mm(whh_sb, h_rhs_fn, g_all, ms, xp_rz_t, xt, bhn_sb):
                """Per m-tile: one PSUM accumulation group whose first matmul
                seeds the bank (identity matmul copying xp for r/z tiles, a
                rank-1 ones matmul adding b_hn for n tiles), followed by the
                4 recurrent k-chunk matmuls."""
                for m in ms:
                    dst = g_all[:, m * B : (m + 1) * B]
                    if m < 8:
                        nc.tensor.matmul(
                            dst, ident_sb[:],
                            xp_rz_t[:, xt, m * B : (m + 1) * B],
                            start=True, stop=False,
                        )
                    else:
                        nc.tensor.matmul(
                            dst,
                            bhn_sb[0:1, (m - 8) * 128 : (m - 7) * 128],
                            ones_bd[0:1, 0:B],
                            start=True, stop=False,
                        )
                    for ki in range(KH):
                        nc.tensor.matmul(
                            dst,
                            whh_sb[:, ki * G3 + m * 128 : ki * G3 + (m + 1) * 128],
                            h_rhs_fn(ki),
                            start=False, stop=(ki == KH - 1),
                        )

            def rec_gate_math(g_all, xp_n_t, xt, h_prev_ap, h_out_ap, tag,
                              phase):
                """phase 0: sig_r + rhn/pren (after r,n tiles); phase 1:
                sig_z + tanh + combine (after z tiles)."""
                if phase == 0:
                    r_sb = wp.tile([128, 4 * B], BF16, tag=f"r{tag}")
                    nc.scalar.activation(r_sb[:], g_all[:, 0 : 4 * B],
                                         AF.Sigmoid)
                    rhn = wp.tile([128, 4 * B], BF16, tag=f"rhn{tag}")
                    nc.vector.tensor_mul(rhn[:], r_sb[:],
                                         g_all[:, 8 * B : 12 * B])
                    pren = wp.tile([128, 4 * B], BF16, tag=f"pren{tag}")
                    nc.vector.tensor_add(pren[:], rhn[:], xp_n_t[:, xt])
                    nneg = wp.tile([128, 4 * B], BF16, tag=f"nneg{tag}")
                    nc.scalar.activation(nneg[:], pren[:], AF.Tanh, scale=-1.0)
                    return nneg
                else:
                    nneg = phase
                    z_sb = wp.tile([128, 4 * B], BF16, tag=f"z{tag}")
                    nc.scalar.activation(z_sb[:], g_all[:, 4 * B : 8 * B],
                                         AF.Sigmoid)
                    zh1 = wp.tile([128, 4 * B], BF16, tag=f"zh1{tag}")
                    nc.gpsimd.tensor_mul(zh1[:], z_sb[:], h_prev_ap)
                    tt = wp.tile([128, 4 * B], BF16, tag=f"tt{tag}")
                    nc.vector.scalar_tensor_tensor(
                        tt[:], z_sb[:], 1.0, nneg[:],
                        op0=ALU.subtract, op1=ALU.mult,
                    )
                    nc.vector.tensor_add(h_out_ap, tt[:], zh1[:])
                    return None

            R_MS = [0, 1, 2, 3]
            N_MS = [8, 9, 10, 11]
            Z_MS = [4, 5, 6, 7]

            def l0_parts(tl):
                prev = L if tl == 0 else tl
                g0 = gp.tile([128, 12 * B], FP32, tag="g0")
                rhs = lambda k, prev=prev: h0seq[:, prev, k * B : (k + 1) * B]
                return dict(
                    g=g0, whh=whh0_sb, rhs=rhs, xprz=xp_rzA, xpn=xp_nA,
                    bhn=bhn0_sb, xt=tl, h_prev=h0seq[:, prev],
                    h_out=h0seq[:, tl + 1], tag="0",
                )

            def l1_parts(tl):
                g1 = gp.tile([128, 12 * B], FP32, tag="g1")
                return dict(
                    g=g1, whh=whh1_sb,
                    rhs=lambda k: h1bf[:, k * B : (k + 1) * B],
                    xprz=xp_rzB, xpn=xp_nB, bhn=bhn1_sb, xt=tl,
                    h_prev=h1bf[:], h_out=h1bf[:], tag="1",
                )

            def emit_rn(p):
                gate_mm(p["whh"], p["rhs"], p["g"], R_MS + N_MS,
                        p["xprz"], p["xt"], p["bhn"])

            def emit_phase0(p):
                p["nneg"] = rec_gate_math(
                    p["g"], p["xpn"], p["xt"], p["h_prev"], p["h_out"],
                    p["tag"], 0)

            def emit_z(p):
                gate_mm(p["whh"], p["rhs"], p["g"], Z_MS,
                        p["xprz"], p["xt"], p["bhn"])

            def emit_phase1(p):
                rec_gate_math(p["g"], p["xpn"], p["xt"], p["h_prev"],
                              p["h_out"], p["tag"], p["nneg"])

            def step_pair(tl, with_l1):
                p0 = l0_parts(tl)
                emit_rn(p0)
                emit_phase0(p0)
                if with_l1:
                    p1 = l1_parts(tl)
                    emit_rn(p1)
                emit_z(p0)
                emit_phase1(p0)
                if with_l1:
                    emit_phase0(p1)
                    emit_z(p1)
                    emit_phase1(p1)

            def l1_step(tl):
                p1 = l1_parts(tl)
                emit_rn(p1)
                emit_phase0(p1)
                emit_z(p1)
                emit_phase1(p1)

            # ---------------- prologue ----------------
            xblks = []
            for ib in range(min(2, NB)):
                xb = xbp.tile([I_IN, BL], BF16, tag="xblk")
                nc.sync.dma_start(xb[:], xT[:, ib * BL : (ib + 1) * BL])
                xblks.append(xb)
            # xproj L0 chunks 0..NXC-2 of block 0, serial (chunk NXC-1 is the
            # in-block tail, fed by the same xblk).
            for ch in range(NXC - 1):
                for m in range(M3):
                    ps = xp_ps.tile([128, NCH], FP32, tag="xps")
                    xproj_mm_l0(xblks[0], ch, m, ps)
                    xproj_ev(ch, m, ps, brz0_sb, bn0_sb, xp_rzA, xp_nA, bool(m & 1))

            # ---------------- main loop ----------------
            for ib in range(NB):
                # DMA for block ib+2 x data (consumed by L0-xproj jobs that
                # start during block ib+1).
                if ib + 2 < NB:
                    xb = xbp.tile([I_IN, BL], BF16, tag="xblk")
                    nc.sync.dma_start(
                        xb[:], xT[:, (ib + 2) * BL : (ib + 3) * BL]
                    )
                    xblks.append(xb)
                xblk_next = xblks[ib + 1] if ib + 1 < NB else None

                sched = make_sched(
                    have_l0=(ib + 1 < NB),
                    have_l1=True,
                    have_l0_tail=True,           # chunk NXC-1 of block ib
                    have_l1_tail=(ib > 0),       # chunk NXC-1 of block ib-1
                )
                # L0 tail jobs read xblk of block ib itself:
                xblk_tail = xblks[ib]

                for tl in range(L):
                    step_pair(tl, with_l1=(ib > 0))
                    for kind, ch, m in sched[tl]:
                        xb = xblk_tail if (kind == 0 and tl < SCH) else xblk_next
                        run_job(kind, ch, m, xb)

            # ---------------- epilogue ----------------
            # L1-xproj tail of the last block, then the last block's L1 steps.
            tailjobs = [(3, NXC - 1, m) for m in range(M3)]
            for tl in range(L):
                if tl >= 1 and tailjobs and tl < SCH:
                    njobs = -(-len(tailjobs) // (SCH - tl))  # ceil
                    for _ in range(njobs):
                        if tailjobs:
                            run_job(*tailjobs.pop(0), None)
                l1_step(tl)

            # ---- head: out = sigmoid(W_fc @ h1 + b_fc), [1, B]
            hps = xp_ps.tile([1, B], FP32, tag="headps", bufs=1)
            for k in range(KH):
                nc.tensor.matmul(
                    hps[:],
                    wfc_sb[:, k : k + 1],
                    h1bf[:, k * B : (k + 1) * B],
                    start=(k == 0),
                    stop=(k == KH - 1),
                )
            osb = pp.tile([1, B], FP32, tag="osb")
            nc.scalar.activation(osb[:], hps[:], AF.Sigmoid, bias=bfc_sb[0:1, 0:1])
            nc.sync.dma_start(out[:], osb[:])
            if debug:
                nc.sync.dma_start(
                    h0dbg[:], h0seq[:, 1:].rearrange("p t b -> p (t b)"))
                nc.sync.dma_start(h1dbg[:], h1bf[:])
                nc.sync.dma_start(
                    xpdbg[:], xp_rzB[:].rearrange("p t b -> p (t b)"))
                nc.sync.dma_start(
                    xpndbg[:], xp_nB[:].rearrange("p t b -> p (t b)"))

    if split_waits:
        split_excess_waits(nc, max_waits=1)
    return nc


def host_pack_inputs(x_shard, W_ih0, W_hh0, b_ih0, b_hh0, W_ih1, W_hh1, b_ih1,
                     b_hh1, W_fc, b_fc):
    """Pack one core's inputs into the DRAM layouts the kernel expects.

    x_shard: [B, T, I] fp32. Returns dict of np arrays (bf16/fp32).
    """
    import numpy as np
    from ml_dtypes import bfloat16

    B, T, _ = x_shard.shape

    def pack_khg(w):  # [3H, K] -> lhsT tiles [128, KH*G3]
        wt = np.ascontiguousarray(w.T)  # [K, 3H]
        k = wt.shape[0] // 128
        return np.ascontiguousarray(
            wt.reshape(k, 128, G3).transpose(1, 0, 2).reshape(128, k * G3)
        ).astype(bfloat16)

    def bias_cols(b):  # [n*128] -> [128, n]
        n = b.shape[0] // 128
        return np.ascontiguousarray(b.reshape(n, 128).T).astype(np.float32)

    # xT: [I, T*B], col = t*B + b
    xT = np.ascontiguousarray(x_shard.transpose(2, 1, 0).reshape(I_IN, T * B))

    return {
        "xT": xT.astype(bfloat16),
        "wih0": np.ascontiguousarray(W_ih0.T).astype(bfloat16),
        "whh0": pack_khg(W_hh0),
        "wih1": pack_khg(W_ih1),
        "whh1": pack_khg(W_hh1),
        "brz0": bias_cols((b_ih0 + b_hh0)[: 2 * H]),
        "bn0": bias_cols(b_ih0[2 * H :]),
        "brz1": bias_cols((b_ih1 + b_hh1)[: 2 * H]),
        "bn1": bias_cols(b_ih1[2 * H :]),
        "bhn0r": np.ascontiguousarray(b_hh0[2 * H :].reshape(1, H)).astype(bfloat16),
        "bhn1r": np.ascontiguousarray(b_hh1[2 * H :].reshape(1, H)).astype(bfloat16),
        "ident": np.eye(128, dtype=np.float32).astype(bfloat16),
        "onesb": np.kron(np.eye(KH, dtype=np.float32),
                         np.ones((1, B), np.float32)).astype(bfloat16),
        "wfc": np.ascontiguousarray(W_fc.reshape(KH, 128).T).astype(bfloat16),
        "bfc": np.array([[b_fc[0]]], dtype=np.float32),
    }


_NC_CACHE = {}


def _get_nc(B, T, L):
    key = (B, T, L)
    if key not in _NC_CACHE:
        tile.TileContext._drain_and_barrier = _patched_drain_and_barrier
        _NC_CACHE[key] = build_gru_nc(B, T, L)
    return _NC_CACHE[key]


def kernel(x, W_ih0, W_hh0, b_ih0, b_hh0, W_ih1, W_hh1, b_ih1, b_hh1, W_fc,
           b_fc):
    """Full-input entry point: shards over 8 cores, returns [B, 1] fp32."""
    from concourse.bass_utils import run_bass_kernel_spmd

    x = np.asarray(x)
    Bfull, T, _ = x.shape
    n_cores = 8
    B = Bfull // n_cores
    L = 100 if T % 100 == 0 else T
    nc = _get_nc(B, T, L)

    wargs = [np.asarray(a) for a in [
        W_ih0, W_hh0, b_ih0, b_hh0, W_ih1, W_hh1, b_ih1, b_hh1, W_fc, b_fc,
    ]]
    in_maps = [
        host_pack_inputs(x[c * B : (c + 1) * B], *wargs) for c in range(n_cores)
    ]
    res = run_bass_kernel_spmd(nc, in_maps, list(range(n_cores)))
    outs = [res.results[c]["out"].reshape(B, 1) for c in range(n_cores)]
    return np.concatenate(outs, axis=0).astype(np.float32)
